# revision 36
# baseline (speedup 1.0000x reference)
"""Trainium2 Bass kernel for nn_ArithmeticUserStateModel.

GRU-based user-state model: B=4096 users x T=256 sequential steps.
Pure data parallel across 8 NeuronCores (512 users per core, weights
replicated). All compute in a transposed layout: feature dims on SBUF
partitions, the 512 local users on the free dim, fp16 on the matmul
path (psum accumulation stays fp32).

Key structure:
- One "state" tile per time-chunk holds [h (0:64) | pe (64:96) |
  obs (96:98)] per step, so the gate pre-activation is a single K=98
  matmul and pred-layer-1 a single K=96 matmul (weights concatenated
  host-side; the 34->64 input projection is also folded into the GRU
  input weights algebraically).
- The batch is split into two independent 256-user half-chains that
  interleave across engines to hide the serial GRU dependency.
"""

import sys

for _p in ("/opt/trn_rl_repo", "/opt/pypackages"):
    if _p not in sys.path:
        sys.path.insert(0, _p)

import numpy as np

import concourse.bacc as bacc
import concourse.tile as tile
from concourse import mybir
from concourse.bass_utils import run_bass_kernel_spmd

B, T = 4096, 256
NCORES = 8
BL = B // NCORES  # 512 users per core
PD, SD, NB = 32, 64, 41
TC = 16   # time chunk (steps per pipelined chunk)
HB = 256  # half-batch columns (two interleaved GRU chains)
F32 = mybir.dt.float32
FP16 = mybir.dt.float16
AF = mybir.ActivationFunctionType
ALU = mybir.AluOpType

_CACHE = {}
TRACE_DIR = "/tmp/bass_trace"


def _build_nc():
    nc = bacc.Bacc(debug=False)

    probs = nc.declare_dram_parameter("probsT", [3, T, BL], FP16,
                                      isOutput=False)
    obs = nc.declare_dram_parameter("obsT", [2, T, BL], FP16, isOutput=False)

    wspec = {
        "w_e1": [3, 32], "b_e1": [32, 1],
        "w_e2": [32, 32], "b_e2": [32, 1],
        "w_grz": [98, 128], "w_xn": [34, 64], "w_hn": [64, 64],
        "b_rz": [128, 1], "b_hn": [64, 1], "b_n": [64, 1],
        "w_p1": [96, 64], "b_p1": [64, 1],
        "w_p2": [64, 64], "b_p2": [64, 1],
        "w_po": [64, 42], "b_po": [42, 1],
    }

    def _wdt(k):
        return FP16 if k.startswith("w_") else F32

    wd = {k: nc.declare_dram_parameter(k, s, _wdt(k), isOutput=False)
          for k, s in wspec.items()}

    out_d = nc.declare_dram_parameter("out", [T, 42, BL], F32, isOutput=True)

    NCH = T // TC

    with tile.TileContext(nc) as tc:
        with (
            tc.tile_pool(name="const", bufs=1) as cpool,
            tc.tile_pool(name="probs", bufs=2) as prob_pool,
            tc.tile_pool(name="states", bufs=3) as st_pool,
            tc.tile_pool(name="work", bufs=3) as work,
            tc.tile_pool(name="outp", bufs=4) as opool,
            tc.tile_pool(name="psum", bufs=1, space="PSUM") as psp,
        ):
            wt = {}
            for k, s in wspec.items():
                if k == "w_xn":
                    # its rhs lives at partitions 64:98 of the state tile;
                    # matmul requires lhsT/rhs base partitions to match
                    t_ = cpool.tile([98, s[1]], _wdt(k), tag=k)
                    nc.sync.dma_start(out=t_[64:98, :], in_=wd[k][:])
                    wt[k] = t_[64:98, :]
                else:
                    t_ = cpool.tile(s, _wdt(k), tag=k)
                    nc.sync.dma_start(out=t_[:], in_=wd[k][:])
                    wt[k] = t_

            mm = nc.tensor.matmul

            # state tile per chunk: rows 0:64 h_{t-1}, 64:96 pe[t],
            # 96:98 obs[t]; slot tl <-> free cols [tl*BL, (tl+1)*BL)
            st_cur = st_pool.tile([98, TC * BL], FP16, tag="states")
            nc.vector.memset(st_cur[0:64, 0:BL], 0.0)  # h_{-1} = 0

            for c in range(NCH):
                t0 = c * TC
                # ---------- Phase A: encoder writes pe/obs into state ----
                probT = prob_pool.tile([3, TC * BL], FP16, tag="probT")
                nc.sync.dma_start(out=probT[:], in_=probs[:, t0:t0 + TC, :])
                nc.sync.dma_start(out=st_cur[96:98, :],
                                  in_=obs[:, t0:t0 + TC, :])
                for tl in range(TC):
                    sl = slice(tl * BL, (tl + 1) * BL)
                    pe1p = psp.tile([64, BL], F32, tag="psenc")
                    mm(pe1p[0:32, :], wt["w_e1"][:], probT[:, sl],
                       start=True, stop=True)
                    pe1s = work.tile([32, BL], FP16, tag="pe1s")
                    nc.scalar.activation(pe1s[:], pe1p[0:32, :], AF.Relu,
                                         bias=wt["b_e1"][:], scale=1.0)
                    pe2p = psp.tile([64, BL], F32, tag="psenc")
                    mm(pe2p[32:64, :], wt["w_e2"][:], pe1s[:],
                       start=True, stop=True)
                    nc.scalar.activation(st_cur[64:96, sl], pe2p[32:64, :],
                                         AF.Relu, bias=wt["b_e2"][:],
                                         scale=1.0)

                st_next = st_pool.tile([98, TC * BL], FP16, tag="states")

                # ---------- Phase B+C: recurrence + prediction ----------
                for tl in range(TC):
                    t = t0 + tl
                    # xn for both halves in one full-width matmul
                    pnx = psp.tile([64, BL], F32, tag="pnx")
                    mm(pnx[:], wt["w_xn"],
                       st_cur[64:98, tl * BL:(tl + 1) * BL],
                       start=True, stop=True)

                    for g in range(2):
                        o = tl * BL + g * HB
                        h_prev = st_cur[0:64, o:o + HB]
                        gtag = "lo" if g == 0 else "hi"
                        ctx = tc.high_priority()
                        ctx.__enter__()

                        # r|z pre-activation: single K=98 matmul over
                        # [h | pe | obs]
                        prz = psp.tile([128, HB], F32, tag="prz" + gtag)
                        mm(prz[:], wt["w_grz"][:], st_cur[0:98, o:o + HB],
                           start=True, stop=True)
                        phn = psp.tile([64, HB], F32, tag="phn" + gtag)
                        mm(phn[:], wt["w_hn"][:], h_prev,
                           start=True, stop=True)

                        rz = work.tile([128, HB], FP16, tag="rz" + gtag)
                        nc.scalar.activation(rz[:], prz[:], AF.Sigmoid,
                                             bias=wt["b_rz"][:], scale=1.0)
                        # rhn = (hn + b_hn) * r
                        rhn = work.tile([64, HB], F32, tag="rhn" + gtag)
                        nc.vector.scalar_tensor_tensor(
                            rhn[:], phn[:], wt["b_hn"][:], rz[0:64, :],
                            op0=ALU.add, op1=ALU.mult)
                        sN = work.tile([64, HB], F32, tag="sN" + gtag)
                        nc.vector.tensor_add(sN[:], rhn[:],
                                             pnx[:, g * HB:g * HB + HB])
                        nT = work.tile([64, HB], FP16, tag="nT" + gtag)
                        nc.scalar.activation(nT[:], sN[:], AF.Tanh,
                                             bias=wt["b_n"][:], scale=1.0)
                        # h' = n + z*(h - n); d parked at partitions 64:128
                        # so the z-multiply has base-aligned SBUF inputs.
                        dhi = work.tile([128, HB], FP16, tag="dhi" + gtag)
                        nc.vector.tensor_sub(dhi[64:128, :], h_prev, nT[:])
                        eT = work.tile([64, HB], FP16, tag="eT" + gtag)
                        nc.vector.tensor_mul(eT[:], rz[64:128, :],
                                             dhi[64:128, :])
                        if tl < TC - 1:
                            h_dst = st_cur[0:64, o + BL:o + BL + HB]
                        else:
                            h_dst = st_next[0:64, g * HB:g * HB + HB]
                        nc.vector.tensor_add(h_dst, nT[:], eT[:])
                        ctx.__exit__(None, None, None)

                    # ---- prediction head for step t ----
                    f1 = work.tile([64, BL], FP16, tag="f1")
                    for g in range(2):
                        o = tl * BL + g * HB
                        pf1 = psp.tile([128, HB], F32, tag="pspred", bufs=2)
                        mm(pf1[64:128, :], wt["w_p1"][:],
                           st_cur[0:96, o:o + HB], start=True, stop=True)
                        nc.vector.tensor_scalar(
                            f1[:, g * HB:g * HB + HB], pf1[64:128, :],
                            wt["b_p1"][:], 0.0, ALU.add, ALU.max)
                    pf2 = psp.tile([64, BL], F32, tag="pspred", bufs=2)
                    mm(pf2[:], wt["w_p2"][:], f1[:], start=True, stop=True)
                    f2 = work.tile([64, BL], FP16, tag="f2")
                    nc.scalar.activation(f2[:], pf2[:], AF.Relu,
                                         bias=wt["b_p2"][:], scale=1.0)
                    po = psp.tile([42, BL], F32, tag="pspred", bufs=2)
                    mm(po[:], wt["w_po"][:], f2[:], start=True, stop=True)
                    ot = opool.tile([42, BL], F32, tag="ot")
                    nc.scalar.activation(ot[:], po[:], AF.Identity,
                                         bias=wt["b_po"][:], scale=1.0)
                    nc.sync.dma_start(out=out_d[t], in_=ot[:])

                st_cur = st_next

    nc.compile()
    return nc


def _fold_weights(inp):
    f = lambda x: np.ascontiguousarray(np.asarray(x), dtype=np.float32)
    h16 = lambda x: np.ascontiguousarray(np.asarray(x, dtype=np.float32),
                                         dtype=np.float16)
    w_x = f(inp["proj_w"]) @ f(inp["gru_wih"])          # (34, 192)
    b_x = f(inp["proj_b"]) @ f(inp["gru_wih"]) + f(inp["gru_bih"])  # (192,)
    b_h = f(inp["gru_bhh"])                             # (192,)
    col = lambda v: np.ascontiguousarray(v.reshape(-1, 1), dtype=np.float32)
    w_grz = np.concatenate([f(inp["gru_whh"][:, :128]), w_x[:, :128]])
    return {
        "w_e1": h16(inp["enc_w1"]), "b_e1": col(f(inp["enc_b1"])),
        "w_e2": h16(inp["enc_w2"]), "b_e2": col(f(inp["enc_b2"])),
        "w_grz": h16(w_grz),                      # (98, 128): [h; pe; obs]
        "w_xn": h16(w_x[:, 128:]),
        "w_hn": h16(inp["gru_whh"][:, 128:]),
        "b_rz": col(b_x[:128] + b_h[:128]),
        "b_hn": col(b_h[128:]), "b_n": col(b_x[128:]),
        "w_p1": h16(inp["pred_w1"]),              # (96, 64): [h; pe]
        "b_p1": col(f(inp["pred_b1"])),
        "w_p2": h16(inp["pred_w2"]), "b_p2": col(f(inp["pred_b2"])),
        "w_po": np.ascontiguousarray(np.concatenate(
            [f(inp["ans_w"]), f(inp["cor_w"])], axis=1), dtype=np.float16),
        "b_po": col(np.concatenate([f(inp["ans_b"]), f(inp["cor_b"])])),
    }


def _run(inputs, trace=False):
    if "nc" not in _CACHE:
        _CACHE["nc"] = _build_nc()
    nc = _CACHE["nc"]

    wts = _fold_weights(inputs)
    f = lambda x: np.asarray(x, dtype=np.float32)
    probs = f(inputs["problems"])
    ansa = f(inputs["answers"])
    cora = f(inputs["corrects"])

    in_maps = []
    for i in range(NCORES):
        s = slice(i * BL, (i + 1) * BL)
        m = {
            "probsT": np.ascontiguousarray(
                probs[s].transpose(2, 1, 0), dtype=np.float16),
            "obsT": np.ascontiguousarray(
                np.stack([ansa[s].T, cora[s].T]), dtype=np.float16),
        }
        m.update(wts)
        in_maps.append(m)

    kw = {}
    if trace:
        import os, shutil
        shutil.rmtree(TRACE_DIR, ignore_errors=True)
        os.makedirs(TRACE_DIR, exist_ok=True)
        kw = {"tmpdir": TRACE_DIR}
    res = run_bass_kernel_spmd(nc, in_maps, core_ids=list(range(NCORES)),
                               trace=trace, **kw)
    outs = [r["out"] for r in res.results]  # each (T, 42, BL)
    ans_logits = np.concatenate(
        [o[:, :41, :].transpose(2, 0, 1) for o in outs], axis=0)
    cor_logits = np.concatenate([o[:, 41, :].T for o in outs], axis=0)
    return (ans_logits, cor_logits), res


def kernel(**inputs):
    (ans_logits, cor_logits), _ = _run(inputs, trace=False)
    return ans_logits, cor_logits


def kernel_traced(**inputs):
    return _run(inputs, trace=True)
